# revision 46
# baseline (speedup 1.0000x reference)
"""Trainium2 Bass kernel for nn_MultiHeadAttention (B=4, S=2048, D=1024, H=16, HD=64).

Sharding: 8 cores = 4 batches (data parallel) x 2 head-groups of 8 heads
(tensor parallel). Each core computes its batch's QKV projections for its 8
heads, full softmax attention, and the partial output projection for its head
group. The host sums the two head-group partials per batch (the hinted
all-reduce, done at gather time) and adds the output/V biases.

Per-core layout (all matmul operands bf16 at N=512; fp32r triggered heavy
HAM clock-throttling at the same cycle count):
  - Host pre-transposes activations to X^T [D, S]; projections stream with
    the contraction dim (features) on partitions.
  - Q^T/K^T [dh, tok] one [128, 2048] tile per head pair (head A partitions
    0:64, head B 64:128); biases added on the PSUM->SBUF move.
  - Scores are computed transposed S^T[k, q]; the heads of a pair alternate
    PE row quadrants (tile_position (0,0)/(64,0), K=64) so LDWEIGHTS of one
    head overlaps the other head's matmul.
  - exp on ACT from 2-bank PSUM [128, 1024] into E^T tiles (scale folded in).
  - PV: lhsT = V' [128, 65] with a ones column, so PSUM row 64 accumulates
    the softmax denominator Z in the same pass.
  - Normalize: Z -> SBUF copy -> DVE reciprocal_approx_fast (the custom DVE
    op misreads PSUM operands, and plain reciprocal is 5x slower) -> GPSIMD
    partition_broadcast -> DVE multiply into O^T tiles.

Schedule: softmax exp is ACT-bound (~17.5us/unit vs ~14us of PE work), and
HAM re-throttles the PE clock to 4/8 whenever the PE micro-idles, so the
sweep is software-pipelined: each weave slot issues [PV of unit u-1] ->
[a projection/out-proj filler chunk] -> [scores group of unit u], keeping
the in-order PE queue saturated with exp-independent work. Q/K projections
are chunked pair-major (pair 0 as preamble, the rest deadline-scheduled as
fillers; unit (qc,p) needs all of kt[p] but only the qc window of qt[p]).
PSUM budget is exactly 8 banks: psp 2x[128,512] + pssc 2x[128,1024] + pso
2x[65,512]. Issue order is deadlock-critical (see in-line comments).
"""

import numpy as np
from contextlib import ExitStack

B, S, D = 4, 2048, 1024
H, HD = 16, 64
NCORES = 8
HPC = H // 2            # heads per core = 8
PAIRS = HPC // 2        # head pairs per core = 4
DH = HPC * HD           # per-core head dims = 512
P = 128
TOK_T = S // P          # 16 token tiles of 128
QC = S // 512           # 4 query chunks of 512
KC = S // P             # 16 key chunks of 128
KCG = KC // 2           # 8 exp groups of 2 key chunks
FC = D // P             # 8 feature chunks of 128

_CACHE = {}


def _bf16(x):
    """Round fp32 -> bf16 (RNE) on host so DRAM inputs are bfloat16.
    Halves DMA bytes and, more importantly, halves PE datapath toggling —
    the fp32r baseline spent most of the attention phase HAM-throttled to
    half clock; bf16 operands keep the PE at full rate."""
    import ml_dtypes
    return np.ascontiguousarray(x, np.float32).astype(ml_dtypes.bfloat16)


def _build(reps=1):
    import concourse.bacc as bacc
    import concourse.mybir as mybir
    import concourse.tile as tile

    dt = mybir.dt
    f32 = dt.float32
    bf16 = dt.bfloat16
    AF = mybir.ActivationFunctionType

    nc = bacc.Bacc("TRN2", target_bir_lowering=False, debug=False)

    xqT = nc.dram_tensor("xqT", [D, S], bf16, kind="ExternalInput")
    xkT = nc.dram_tensor("xkT", [D, S], bf16, kind="ExternalInput")
    xvT = nc.dram_tensor("xvT", [D, S], bf16, kind="ExternalInput")
    wq = nc.dram_tensor("wq", [D, DH], bf16, kind="ExternalInput")
    wk = nc.dram_tensor("wk", [D, DH], bf16, kind="ExternalInput")
    wv = nc.dram_tensor("wv", [D, DH], bf16, kind="ExternalInput")
    wo = nc.dram_tensor("wo", [DH, D], bf16, kind="ExternalInput")
    biases = nc.dram_tensor("biases", [P, 3 * PAIRS], f32, kind="ExternalInput")
    out = nc.dram_tensor("out", [S, D], f32, kind="ExternalOutput")

    QCC = 4          # query chunks of 512
    QW = S // QCC    # 512
    KPG = 2          # key tiles per exp group
    NG = KC // KPG   # 8 exp groups per (head, qc)

    def mmr(psum, lhsT, rhs, **kw):
        nc.tensor.matmul(psum, lhsT, rhs, **kw)

    with tile.TileContext(nc, pool_alloc_mode="queue") as tc, ExitStack() as ctx:
        # ---- persistent pools ----
        qt_pool = ctx.enter_context(tc.tile_pool(name="qt", bufs=PAIRS))
        kt_pool = ctx.enter_context(tc.tile_pool(name="kt", bufs=PAIRS))
        vpr_pool = ctx.enter_context(tc.tile_pool(name="vpr", bufs=TOK_T))
        ot_pool = ctx.enter_context(tc.tile_pool(name="ot", bufs=8))
        zr_pool = ctx.enter_context(tc.tile_pool(name="zr", bufs=4))
        zb_pool = ctx.enter_context(tc.tile_pool(name="zb", bufs=4))
        bias_pool = ctx.enter_context(tc.tile_pool(name="bias", bufs=1))
        dram_pool = ctx.enter_context(tc.tile_pool(name="dram", bufs=1, space="DRAM"))
        psp = ctx.enter_context(tc.tile_pool(name="psp", bufs=2, space="PSUM"))
        pssc = ctx.enter_context(tc.tile_pool(name="pssc", bufs=2, space="PSUM"))
        pso = ctx.enter_context(tc.tile_pool(name="pso", bufs=2, space="PSUM"))

        SCALE = 1.0 / float(np.sqrt(HD))
        for rep in range(reps):
          qt_t = [qt_pool.tile([P, S], bf16, name=f"qt_{rep}_{p}", tag="qt")
                  for p in range(PAIRS)]
          kt_t = [kt_pool.tile([P, S], bf16, name=f"kt_{rep}_{p}", tag="kt")
                  for p in range(PAIRS)]
          bias_t = bias_pool.tile([P, 3 * PAIRS], f32, name=f"bias_{rep}", tag="bias")
          nc.sync.dma_start(bias_t[:], biases[:])
          bq_t = {p: bias_t[:, p:p + 1] for p in range(PAIRS)}
          bk_t = {p: bias_t[:, PAIRS + p:PAIRS + p + 1] for p in range(PAIRS)}
          bv_t = {p: bias_t[:, 2 * PAIRS + p:2 * PAIRS + p + 1] for p in range(PAIRS)}

          # ---- attention helpers ----
          def alloc_et(p, qc):
              et = {}
              for hh in range(2):
                  for quarter in range(4):
                      et[(hh, quarter)] = et_pool.tile(
                          [P, 4 * QW], bf16,
                          name=f"et_{rep}_{p}_{qc}_{hh}_{quarter}", tag="et")
              return et

          def scores_exp_group(p, qc, g, et):
              half, goff = g // 2, (g % 2) * KPG * QW  # quarter idx, offset
              psA = pssc.tile([P, KPG * QW], f32,
                              name=f"scA_{rep}_{p}_{qc}_{g}", tag="pssc")
              psB = pssc.tile([P, KPG * QW], f32,
                              name=f"scB_{rep}_{p}_{qc}_{g}", tag="pssc")
              for j in range(KPG):
                  kc = g * KPG + j
                  nc.tensor.matmul(
                      psA[:, j * QW:(j + 1) * QW],
                      kt_t[p][0:64, kc * P:(kc + 1) * P],
                      qt_t[p][0:64, qc * QW:(qc + 1) * QW],
                      start=True, stop=True, tile_position=(0, 0))
                  nc.tensor.matmul(
                      psB[:, j * QW:(j + 1) * QW],
                      kt_t[p][64:128, kc * P:(kc + 1) * P],
                      qt_t[p][64:128, qc * QW:(qc + 1) * QW],
                      start=True, stop=True, tile_position=(64, 0))
              nc.scalar.activation(et[(0, half)][:, goff:goff + KPG * QW],
                                   psA[:], AF.Exp, scale=SCALE)
              nc.scalar.activation(et[(1, half)][:, goff:goff + KPG * QW],
                                   psB[:], AF.Exp, scale=SCALE)

          def scores_exp(p, qc):
              et = alloc_et(p, qc)
              for g in range(NG):
                  scores_exp_group(p, qc, g, et)
              return et

          def pv_chunk(p, qc, et, po_pair, kcs):
              poA, poB = po_pair
              for kc in kcs:
                  half, koff = kc // 4, (kc % 4) * QW
                  cA = (2 * p) * (HD + 1)
                  cB = (2 * p + 1) * (HD + 1)
                  mmr(poA[:], vpr_t[kc][:, cA:cA + HD + 1],
                      et[(0, half)][:, koff:koff + QW],
                      start=(kc == 0), stop=(kc == KC - 1))
                  mmr(poB[:], vpr_t[kc][:, cB:cB + HD + 1],
                      et[(1, half)][:, koff:koff + QW],
                      start=(kc == 0), stop=(kc == KC - 1))

          def normalize(p, qc, po_pair):
              # 1/Z via the fast DVE approx (18 bits, Z ~ O(S) so no edge
              # cases); issue both recips, then both broadcasts, then both
              # muls so the DVE->GPSIMD->DVE chain pipelines across heads.
              # The V bias is folded into the host-side epilogue (softmax
              # rows sum to 1, so + bv commutes past attention and Wo).
              poA, poB = po_pair
              ot_t = ot_pool.tile([P, QW], bf16, name=f"ot_{rep}_{p}_{qc}", tag="ot")
              zr, zb = {}, {}
              for hh, po in ((0, poA), (1, poB)):
                  # approx_fast misreads PSUM operands -> bounce Z through SBUF
                  zc = zr_pool.tile([1, QW], f32,
                                    name=f"zc_{rep}_{p}_{qc}_{hh}", tag="zr")
                  nc.vector.tensor_copy(zc[:], po[64:65, :])
                  zr[hh] = zr_pool.tile([1, QW], f32,
                                        name=f"zr_{rep}_{p}_{qc}_{hh}", tag="zr")
                  nc.vector.reciprocal_approx_fast(zr[hh][:], zc[:])
              for hh in range(2):
                  zb[hh] = zb_pool.tile([64, QW], f32,
                                        name=f"zb_{rep}_{p}_{qc}_{hh}", tag="zb")
                  nc.gpsimd.partition_broadcast(zb[hh][:], zr[hh][:])
              for hh, po in ((0, poA), (1, poB)):
                  nc.vector.tensor_mul(ot_t[hh * 64:(hh + 1) * 64, :],
                                       po[0:64, :], zb[hh][:])
              return ot_t

          def attention(p, qc):
              et = scores_exp(p, qc)
              poA = pso.tile([65, QW], f32, name=f"poA_{rep}_{p}_{qc}", tag="pso")
              poB = pso.tile([65, QW], f32, name=f"poB_{rep}_{p}_{qc}", tag="pso")
              pv_chunk(p, qc, et, (poA, poB), range(KC))
              return normalize(p, qc, (poA, poB))

          vpr_t = {}
          # ---- V projection into resident V' tiles (runs first) ----
          with tc.tile_pool(name="wvp0", bufs=FC) as wv_pool0, \
               tc.tile_pool(name="xsv0", bufs=10) as xsv_pool0:
              wv_t0 = []
              for f in range(FC):
                  t = wv_pool0.tile([P, 512], bf16, name=f"wv0_{rep}_{f}", tag="wv0")
                  nc.sync.dma_start(t[:], wv[f * P:(f + 1) * P, :])
                  wv_t0.append(t)
              for tcg in range(TOK_T // 4):
                  xv_t = []
                  for f in range(FC):
                      t = xsv_pool0.tile([P, 512], bf16,
                                         name=f"xv0_{rep}_{tcg}_{f}", tag="xsv0")
                      nc.sync.dma_start(t[:], xvT[f * P:(f + 1) * P,
                                                  tcg * 512:(tcg + 1) * 512])
                      xv_t.append(t)
                  for tl in range(4):
                      tci = tcg * 4 + tl
                      ps = psp.tile([P, 512], f32, name=f"psv_{rep}_{tci}", tag="psp")
                      for f in range(FC):
                          mmr(ps[:], xv_t[f][:, tl * P:(tl + 1) * P], wv_t0[f][:],
                              start=(f == 0), stop=(f == FC - 1))
                      vt = vpr_pool.tile([P, HPC * (HD + 1)], bf16,
                                         name=f"vpr_{rep}_{tci}", tag="vpr")
                      v3 = vt.rearrange("p (h c) -> p h c", c=HD + 1)
                      nc.gpsimd.memset(v3[:, :, HD:HD + 1], 1.0)
                      nc.vector.tensor_copy(v3[:, :, 0:HD],
                                            ps.rearrange("p (h c) -> p h c", c=HD))
                      vpr_t[tci] = vt

          # ---- software-pipelined projection + attention sweep ----
          # HAM re-throttles the PE clock to 4/8 whenever the PE micro-idles,
          # so the schedule keeps the in-order PE queue saturated: the Q/K
          # projections are chunked PAIR-MAJOR (pair 0 as a short preamble,
          # the rest as deadline-scheduled fillers), and each weave slot
          # issues [PV(u-1) chunk] -> [filler chunk] -> [scores(u) group] so
          # nothing queued ahead of ready work waits on the ACT-paced exp.
          with tc.tile_pool(name="xs", bufs=16) as xs_pool, \
               tc.tile_pool(name="wqk", bufs=2 * FC) as wqk_pool, \
               tc.tile_pool(name="et", bufs=12) as et_pool, \
               tc.tile_pool(name="wop", bufs=2 * PAIRS) as wo_pool, \
               tc.tile_pool(name="os", bufs=3) as os_pool:
              wqk_t = {}
              for nm, wT in (("q", wq), ("k", wk)):
                  for f in range(FC):
                      t = wqk_pool.tile([P, 512], bf16,
                                        name=f"w{nm}_{rep}_{f}", tag="wqk")
                      nc.sync.dma_start(t[:], wT[f * P:(f + 1) * P, :])
                      wqk_t[(nm, f)] = t

              def qk_dma(p, tc4, nm):
                  xT = {"q": xqT, "k": xkT}[nm]
                  x_t = []
                  for f in range(FC):
                      t = xs_pool.tile([P, 512], bf16,
                                       name=f"x{nm}_{rep}_{p}_{tc4}_{f}", tag="xs")
                      nc.sync.dma_start(t[:], xT[f * P:(f + 1) * P,
                                                 tc4 * 512:(tc4 + 1) * 512])
                      x_t.append(t)
                  return x_t

              def qk_mms(p, tc4, nm, x_t):
                  dst = {"q": qt_t, "k": kt_t}[nm]
                  b_t = {"q": bq_t, "k": bk_t}[nm]
                  ps = psp.tile([P, 512], f32,
                                name=f"ps{nm}_{rep}_{p}_{tc4}", tag="psp")
                  for f in range(FC):
                      mmr(ps[:], wqk_t[(nm, f)][:, p * P:(p + 1) * P], x_t[f][:],
                          start=(f == 0), stop=(f == FC - 1))
                  nc.vector.tensor_scalar_add(
                      dst[p][:, tc4 * 512:(tc4 + 1) * 512], ps[:], b_t[p][:])

              def prep(spec):
                  """Issue a filler chunk's DMAs; return its PE work closures."""
                  p, tc4, nm = spec
                  x_t = qk_dma(p, tc4, nm)
                  return [lambda p_=p, t4_=tc4, nm_=nm, x_=x_t: qk_mms(p_, t4_, nm_, x_)]

              # per-pair chunk order: all 4 K chunks, then Q(qc0); Q(qc1..3)
              # are deferred until just before the unit that consumes them.
              def pair_chunks_early(p):
                  return ([(p, t4, "k") for t4 in range(QC)] + [(p, 0, "q")])

              # preamble: pair 0 ready before unit (qc0, p0) scores start
              for spec in pair_chunks_early(0):
                  qk_mms(*spec, qk_dma(*spec))

              n_units = QCC * PAIRS
              fillers = {i: [] for i in range(n_units)}
              for p in range(1, PAIRS):
                  fillers[p - 1].extend(pair_chunks_early(p))
              for qc in range(1, QCC):
                  for p in range(PAIRS):
                      fillers[max(0, qc * PAIRS + p - 2)].append((p, qc, "q"))
              wo_t = {}
              for p in range(PAIRS):
                  for dc in range(2):
                      t = wo_pool.tile([P, 512], bf16,
                                       name=f"wo_{rep}_{p}_{dc}", tag="wo")
                      nc.sync.dma_start(t[:], wo[p * P:(p + 1) * P,
                                                 dc * 512:(dc + 1) * 512])
                      wo_t[(p, dc)] = t

              ots = {qc: {} for qc in range(QCC)}
              outq = []

              def outproj_chunk(qc, tl, dc):
                  tci = qc * (QW // P) + tl
                  ps = psp.tile([P, 512], f32,
                                name=f"pout_{rep}_{tci}_{dc}", tag="psp")
                  for pp in range(PAIRS):
                      mmr(ps[:], ots[qc][pp][:, tl * P:(tl + 1) * P],
                          wo_t[(pp, dc)][:],
                          start=(pp == 0), stop=(pp == PAIRS - 1))
                  ost = os_pool.tile([P, 512], f32,
                                     name=f"os_{rep}_{tci}_{dc}", tag="os")
                  nc.vector.tensor_copy(ost[:], ps[:])
                  nc.sync.dma_start(out[tci * P:(tci + 1) * P,
                                        dc * 512:(dc + 1) * 512], ost[:])

              units = [(qc, p) for qc in range(QCC) for p in range(PAIRS)]
              prev = None           # (qc, p, et, (poA, poB)) of unit in PV stage
              to_dma = []           # proj chunk specs due, DMA not yet issued
              pending = []          # [(spec, x_tiles)] chunks with DMA in flight
              for ui, (qc, p) in enumerate(units):
                  to_dma.extend(fillers[ui])
                  et_u = alloc_et(p, qc)
                  for g in range(NG):
                      # stage <=2 filler chunks' x tiles at a time (pools hold
                      # 16 tiles; deeper staging would block the DMA queues)
                      def top_up():
                          while to_dma and len(pending) < 2:
                              pending.extend(prep(to_dma.pop(0)))
                      top_up()
                      # Ordering is deadlock-critical on the in-order PE
                      # queue: scB(u,g) needs the pssc slot held by
                      # B-exp(g-1), and for ODD g that exp waits on this
                      # iteration's PV chunk releasing an et quarter of u-1,
                      # so PV must precede scores (PV-after-scores there
                      # hangs the device). For EVEN g the quarter was freed
                      # by groups g-2/g-1, so scores may go first — letting
                      # the PE enter each unit without waiting behind PV's
                      # PSUM-release chain. Fillers go before scores so the
                      # ACT-paced scores never strand ready work behind them.
                      if prev is not None and g % 2 == 1:
                          pv_chunk(prev[1], prev[0], prev[2], prev[3],
                                   (2 * g, 2 * g + 1))
                      due = len(pending) + len(to_dma)
                      budget = (due + NG - 1 - g) // (NG - g)
                      for _ in range(budget):
                          if not pending:
                              break
                          pending.pop(0)()
                          top_up()
                      if not budget and outq:
                          outq.pop(0)()
                      scores_exp_group(p, qc, g, et_u)
                      if prev is not None and g % 2 == 0:
                          pv_chunk(prev[1], prev[0], prev[2], prev[3],
                                   (2 * g, 2 * g + 1))
                  if prev is not None:
                      pq, pp_ = prev[0], prev[1]
                      ots[pq][pp_] = normalize(pp_, pq, prev[3])
                      if len(ots[pq]) == PAIRS:
                          outq.extend(
                              (lambda qc_=pq, tl_=tl, dc_=dc:
                               outproj_chunk(qc_, tl_, dc_))
                              for tl in range(QW // P) for dc in range(2))
                  poA = pso.tile([65, QW], f32,
                                 name=f"poA_{rep}_{p}_{qc}", tag="pso")
                  poB = pso.tile([65, QW], f32,
                                 name=f"poB_{rep}_{p}_{qc}", tag="pso")
                  prev = (qc, p, et_u, (poA, poB))
              # drain: PV + normalize of the last unit, then leftover out-proj
              for g in range(NG):
                  pv_chunk(prev[1], prev[0], prev[2], prev[3],
                           (2 * g, 2 * g + 1))
                  if outq:
                      outq.pop(0)()
              ots[prev[0]][prev[1]] = normalize(prev[1], prev[0], prev[3])
              outq.extend(
                  (lambda qc_=prev[0], tl_=tl, dc_=dc:
                   outproj_chunk(qc_, tl_, dc_))
                  for tl in range(QW // P) for dc in range(2))
              for c in outq:
                  c()
    nc.compile()
    return nc


def _get_nc(reps=1):
    if reps not in _CACHE:
        _CACHE[reps] = _build(reps)
    return _CACHE[reps]


def _in_maps(inputs):
    f = np.float32
    maps = []
    for c in range(NCORES):
        b, g = c // 2, c % 2
        hs = slice(g * HPC, (g + 1) * HPC)
        maps.append({
            "xqT": _bf16(np.asarray(inputs["inputs_q"][b], f).T),
            "xkT": _bf16(np.asarray(inputs["inputs_k"][b], f).T),
            "xvT": _bf16(np.asarray(inputs["inputs_v"][b], f).T),
            "wq": _bf16(np.asarray(inputs["Wq"], f)[:, hs, :].reshape(D, DH)),
            "wk": _bf16(np.asarray(inputs["Wk"], f)[:, hs, :].reshape(D, DH)),
            "wv": _bf16(np.asarray(inputs["Wv"], f)[:, hs, :].reshape(D, DH)),
            "wo": _bf16(np.asarray(inputs["Wo"], f)[hs].reshape(DH, D)),
            "biases": np.stack(
                [np.asarray(inputs[nm], f)[hs].reshape(DH)[p * P:(p + 1) * P]
                 for nm in ("bq", "bk", "bv") for p in range(PAIRS)], axis=1).copy(),
        })
    return maps


def run_sharded(inputs, **kw):
    """Compile/run on all 8 cores; returns (full_output, BassKernelResults)."""
    from concourse.bass_utils import run_bass_kernel_spmd
    nc = _get_nc()
    res = run_bass_kernel_spmd(nc, _in_maps(inputs), core_ids=list(range(NCORES)), **kw)
    # Epilogue: the kernel returns sum_h softmax(s_h) V_h Wo_h per head-group;
    # the V bias rides along as bv @ Wo (softmax rows sum to 1), plus bo.
    bo = np.asarray(inputs["bo"], np.float32)
    bv = np.asarray(inputs["bv"], np.float32)
    wo_f = np.asarray(inputs["Wo"], np.float32)
    bias_full = bo + np.einsum("hd,hdo->o", bv, wo_f)
    full = np.empty((B, S, D), np.float32)
    for b in range(B):
        full[b] = (res.results[2 * b]["out"].astype(np.float32)
                   + res.results[2 * b + 1]["out"].astype(np.float32) + bias_full)
    return full, res


def kernel(**inputs) -> np.ndarray:
    full, _ = run_sharded(inputs)
    return full

